# revision 22
# baseline (speedup 1.0000x reference)
"""GRU-cell-variant kernel for Trainium2, data-parallel over batch on 8 cores.

Reference (per batch row b, hidden size H=1024):
    gates = sigmoid(x @ W_ih + b_ih + h @ W_hh + b_hh)   # [B, 2H]
    z, r  = gates[:, :H], gates[:, H:]
    cand  = tanh(x @ W_c + b_c + r * (h @ W_hc + b_hc))
    out   = (1 - z) * h + z * cand

Design:
  - 8-way batch shard (1024 rows/core), weights replicated. No collectives.
  - Everything on-chip is computed TRANSPOSED: out.T[o, b], so weight tiles
    [k, o] are the stationary operand and host-pre-transposed x.T / h.T the
    moving operand; biases are per-partition (free on the ACT engine).
  - Mixed precision tuned against the 2e-2 rel-err budget (measured on the
    harness inputs): z/r gate matmuls and h@W_hc run in fp8 e4m3 with
    perf_mode=DoubleRow (2 contraction rows per PE cell per cycle);
    x@W_c — the most error-sensitive matmul — stays fp16. Weights are
    pre-scaled by 128 and activations by 8 so e4m3's normal range is used;
    the 1/1024 de-scale folds into the ACT-engine activation scale.
    End-to-end rel err ~1.45e-2 vs 2.05e-2 for all-fp8.
  - Per j, all six DoubleRow gate/hc groups run back-to-back and the two
    fp16 x@W_c groups follow: switching the PE from fp16 back to DoubleRow
    costs ~0.4-0.6us (the DR LDWEIGHTS can't be prefetched during an fp16
    stream), so the order pays that once per j instead of twice.
  - PSUM accumulates fp32; elementwise + residual in fp32 (h residual fp16).
  - ~36 warm-up matmuls on a zeroed SBUF tile run during the DMA preamble so
    the PE HAM clock-gate is already at full rate when real matmuls start.
  - Host packs weights/activations into the exact SBUF layouts so every DMA
    is a dense 2D copy with wide per-partition lines; the cold-start DMAs
    are ordered by PE need time across both HWDGE rings.
"""

import numpy as np
import ml_dtypes

import concourse.bass as bass
import concourse.mybir as mybir
import concourse.tile as tile
from concourse import bacc
from concourse.bass_utils import run_bass_kernel_spmd

N_CORES = 8
B = 8192
H = 1024
BL = B // N_CORES  # batch rows per core
P = 128
KC = H // P  # 8 contraction chunks of 128 per 1024-wide operand
NJ = H // P  # 8 hidden-dim tiles
NB = BL // 512  # 2 moving halves of 512 batch columns
HB = 4096  # fp8 elems per half in [p, hb*4096 + kc*512 + bb] layouts

SW = 128.0  # weight fp8 scale
SA = 8.0  # activation fp8 scale
INV = 1.0 / (SW * SA)  # psum de-scale

N_WARM = 56  # PE warm-up matmuls (N=128) during the DMA preamble

F8 = mybir.dt.float8e4
F16 = mybir.dt.float16
F32 = mybir.dt.float32
AF = mybir.ActivationFunctionType
ALU = mybir.AluOpType
DR = mybir.MatmulPerfMode.DoubleRow

E4M3 = ml_dtypes.float8_e4m3

_CACHE = {}


def _build_program():
    nc = bacc.Bacc(
        "TRN2",
        target_bir_lowering=False,
        debug=False,
        enable_asserts=False,
        num_devices=N_CORES,
    )

    # DRAM inputs, packed on the host (see _pack_* below).
    # x8/h8:  [p, hb*4096 + kc*512 + bb] = 8*a[hb*512+bb, kc*128+p]   (fp8)
    # x16:    same layout, fp16, unscaled (moving operand of x@W_c)
    # h16:    [p, hb*4096 + j*512 + bb] = h[hb*512+bb, j*128+p]       (fp16)
    # Wg8:    [p, t*2048 + kc*128 + jj] = 128*Wg_full[kc*128+p, t*128+jj]
    #          t in [0,16): gate output tile; kc in [0,16): contraction [x;h]
    # Whc8:   [p, j*1024 + kc*128 + jj] = 128*W_hc[kc*128+p, j*128+jj]
    # Wc16:   same transform of W_c, fp16, unscaled
    # bg:     [p, t] = (b_ih+b_hh)[t*128+p]; bc analogous; bhcs = 1024*b_hc
    x8 = nc.dram_tensor("x8", [P, 2 * HB], F8, kind="ExternalInput").ap()
    h8 = nc.dram_tensor("h8", [P, 2 * HB], F8, kind="ExternalInput").ap()
    x16 = nc.dram_tensor("x16", [P, 2 * HB], F16, kind="ExternalInput").ap()
    h16 = nc.dram_tensor("h16", [P, 2 * HB], F16, kind="ExternalInput").ap()
    Wg8 = nc.dram_tensor("Wg8", [P, 16 * 2048], F8, kind="ExternalInput").ap()
    Whc8 = nc.dram_tensor("Whc8", [P, NJ * H], F8, kind="ExternalInput").ap()
    Wc16 = nc.dram_tensor("Wc16", [P, NJ * H], F16, kind="ExternalInput").ap()
    Wc8j0 = nc.dram_tensor("Wc8j0", [P, H], F8, kind="ExternalInput").ap()
    bg = nc.dram_tensor("bg", [P, 16], F32, kind="ExternalInput").ap()
    bc = nc.dram_tensor("bc", [P, NJ], F32, kind="ExternalInput").ap()
    bhcs = nc.dram_tensor("bhcs", [P, NJ], F32, kind="ExternalInput").ap()
    outT = nc.dram_tensor("outT", [P, NJ * BL], F16, kind="ExternalOutput").ap()

    with tile.TileContext(nc) as tc:
        with (
            tc.tile_pool(name="const", bufs=1) as cpool,
            tc.tile_pool(name="wg", bufs=4) as wgpool,
            tc.tile_pool(name="wsm", bufs=4) as wsmpool,
            tc.tile_pool(name="psum", bufs=8, space="PSUM") as ppool,
            tc.tile_pool(name="gates", bufs=8) as gpool,
            tc.tile_pool(name="work", bufs=18) as wpool,
        ):
            bg_sb = cpool.tile([P, 16], F32, tag="bg")
            bc_sb = cpool.tile([P, NJ], F32, tag="bc")
            bhc_sb = cpool.tile([P, NJ], F32, tag="bhc")

            # Resident activations.
            x8_sb = cpool.tile([P, 2 * HB], F8, tag="x8")
            h8_sb = cpool.tile([P, 2 * HB], F8, tag="h8")
            x16_sb = cpool.tile([P, 2 * HB], F16, tag="x16")
            h16_sb = cpool.tile([P, 2 * HB], F16, tag="h16")

            # --- PE warm-up: run dummy matmuls on a zeroed tile while the
            # input DMAs stream, so the HAM clock-gate reaches full rate
            # before the first real matmul issues.
            warm = cpool.tile([P, P], F16, tag="warm")
            nc.vector.memset(warm[:], 0.0)
            pdum = ppool.tile([P, 512], F32, tag="ps")
            for _ in range(N_WARM):
                nc.tensor.matmul(pdum[:, 0:P], lhsT=warm[:], rhs=warm[:])

            def dr2(ap2d):
                # [p, 2*w] slice -> [p, 2, w] DoubleRow AP
                return ap2d.rearrange("p (k m) -> p k m", k=2)

            def gate_matmuls(psum, w_sb, hb, half=None):
                # accumulate over [x;h]: pairs 0-3 read x8, 4-7 read h8.
                # half='x'/'h' emits only that part of the accumulation
                # group (cold start: x-only work runs while h still streams).
                cs = range(KC) if half is None else (
                    range(KC // 2) if half == "x" else range(KC // 2, KC)
                )
                for c in cs:
                    src = x8_sb if c < 4 else h8_sb
                    o = hb * HB + ((2 * c) % KC) * 512
                    nc.tensor.matmul(
                        psum[:],
                        lhsT=dr2(w_sb[:, 2 * c * P : (2 * c + 2) * P]),
                        rhs=dr2(src[:, o : o + 1024]),
                        start=(c == 0),
                        stop=(c == KC - 1),
                        perf_mode=DR,
                    )

            def hc_matmuls(psum, w_sb, hb):
                for c in range(KC // 2):
                    o = hb * HB + 2 * c * 512
                    nc.tensor.matmul(
                        psum[:],
                        lhsT=dr2(w_sb[:, 2 * c * P : (2 * c + 2) * P]),
                        rhs=dr2(h8_sb[:, o : o + 1024]),
                        start=(c == 0),
                        stop=(c == KC // 2 - 1),
                        perf_mode=DR,
                    )

            def xc_dr_matmuls(psum, w_sb, hb):
                # j0 cold-start path: x@W_c in fp8 DoubleRow off resident x8,
                # so j0 needs neither x16 nor an fp16<->DR mode switch.
                for c in range(KC // 2):
                    o = hb * HB + 2 * c * 512
                    nc.tensor.matmul(
                        psum[:],
                        lhsT=dr2(w_sb[:, 2 * c * P : (2 * c + 2) * P]),
                        rhs=dr2(x8_sb[:, o : o + 1024]),
                        start=(c == 0),
                        stop=(c == KC // 2 - 1),
                        perf_mode=DR,
                    )

            def xc_matmuls(psum, w_sb, hb):
                for kc in range(KC):
                    o = hb * HB + kc * 512
                    nc.tensor.matmul(
                        psum[:],
                        lhsT=w_sb[:, kc * P : (kc + 1) * P],
                        rhs=x16_sb[:, o : o + 512],
                        start=(kc == 0),
                        stop=(kc == KC - 1),
                    )

            for j in range(NJ):
                wz = wgpool.tile([P, 2048], F8, tag="wg")
                wr = wgpool.tile([P, 2048], F8, tag="wg")
                whc_w = wsmpool.tile([P, H], F8, tag="whc")
                if j == 0:
                    wc_w = wsmpool.tile([P, H], F8, tag="whc", name="wc8_0")
                else:
                    wc_w = wsmpool.tile([P, H], F16, tag="wc")
                if j == 0:
                    # Cold-start feed, ordered by PE need time. sync ring:
                    # gate activations (both halves). ACT ring: j0 weights,
                    # constants, residual h16 j0.
                    HB2 = HB // 2
                    nc.sync.dma_start(x8_sb[:, 0:HB2], x8[:, 0:HB2])
                    nc.sync.dma_start(x8_sb[:, HB2:HB], x8[:, HB2:HB])
                    nc.sync.dma_start(h8_sb[:, 0:HB2], h8[:, 0:HB2])
                    nc.sync.dma_start(h8_sb[:, HB2:HB], h8[:, HB2:HB])
                    nc.sync.dma_start(x8_sb[:, HB : 2 * HB], x8[:, HB : 2 * HB])
                    nc.sync.dma_start(h8_sb[:, HB : 2 * HB], h8[:, HB : 2 * HB])
                    nc.scalar.dma_start(wz[:, 0:1024], Wg8[:, 0:1024])
                    nc.scalar.dma_start(wz[:, 1024:2048], Wg8[:, 1024:2048])
                    nc.scalar.dma_start(bg_sb[:], bg[:])
                    nc.scalar.dma_start(wr[:, 0:1024], Wg8[:, NJ * 2048 : NJ * 2048 + 1024])
                    nc.scalar.dma_start(wr[:, 1024:2048], Wg8[:, NJ * 2048 + 1024 : NJ * 2048 + 2048])
                    nc.scalar.dma_start(whc_w[:], Whc8[:, 0:H])
                    nc.scalar.dma_start(bc_sb[:], bc[:])
                    nc.scalar.dma_start(bhc_sb[:], bhcs[:])
                    nc.scalar.dma_start(wc_w[:], Wc8j0[:])
                    nc.scalar.dma_start(h16_sb[:, 0:512], h16[:, 0:512])
                    nc.scalar.dma_start(h16_sb[:, HB : HB + 512], h16[:, HB : HB + 512])
                elif j == 1:
                    # split j=1 weights across the two rings
                    nc.sync.dma_start(wz[:], Wg8[:, 1 * 2048 : 2 * 2048])
                    nc.scalar.dma_start(wr[:], Wg8[:, (NJ + 1) * 2048 : (NJ + 2) * 2048])
                    nc.sync.dma_start(whc_w[:], Whc8[:, H : 2 * H])
                    nc.scalar.dma_start(wc_w[:], Wc16[:, H : 2 * H])
                else:
                    nc.sync.dma_start(wz[:], Wg8[:, j * 2048 : (j + 1) * 2048])
                    nc.sync.dma_start(wr[:], Wg8[:, (NJ + j) * 2048 : (NJ + j + 1) * 2048])
                    nc.sync.dma_start(whc_w[:], Whc8[:, j * H : (j + 1) * H])
                    nc.sync.dma_start(wc_w[:], Wc16[:, j * H : (j + 1) * H])
                if j > 0:
                    # residual-path h (fp16) rides the ACT ring, per (j, half)
                    for hb in range(2):
                        o = hb * HB + j * 512
                        nc.scalar.dma_start(h16_sb[:, o : o + 512], h16[:, o : o + 512])
                if j == 1:
                    # x16 is first consumed by j1's px groups — load it after
                    # j1's weights so it doesn't block them in the queues
                    HB2 = HB // 2
                    nc.sync.dma_start(x16_sb[:, 0:HB2], x16[:, 0:HB2])
                    nc.scalar.dma_start(x16_sb[:, HB2:HB], x16[:, HB2:HB])
                    nc.sync.dma_start(x16_sb[:, HB : HB + HB2], x16[:, HB : HB + HB2])
                    nc.scalar.dma_start(x16_sb[:, HB + HB2 : 2 * HB], x16[:, HB + HB2 : 2 * HB])

                # --- PE: all DoubleRow groups back-to-back, then the two
                # fp16 x@W_c groups (one DR<-fp16 mode switch per j).
                pz = [None, None]
                pr = [None, None]
                ph = [None, None]
                px = [None, None]
                if j == 0:
                    # Cold start: per half, emit the gate x-halves first and
                    # bridge the wait for the h8 stream with dummy matmuls
                    # (separate PSUM bank) so the PE never idles long enough
                    # for the HAM clock-gate to re-throttle.
                    for hb in range(NB):
                        pz[hb] = ppool.tile([P, 512], F32, tag="ps", name=f"pz{hb}")
                        gate_matmuls(pz[hb], wz, hb, half="x")
                        pr[hb] = ppool.tile([P, 512], F32, tag="ps", name=f"pr{hb}")
                        gate_matmuls(pr[hb], wr, hb, half="x")
                        for _ in range(26 if hb == 0 else 15):
                            nc.tensor.matmul(pdum[:, 0:P], lhsT=warm[:], rhs=warm[:])
                        gate_matmuls(pz[hb], wz, hb, half="h")
                        gate_matmuls(pr[hb], wr, hb, half="h")
                        ph[hb] = ppool.tile([P, 512], F32, tag="ps", name=f"ph{hb}")
                        hc_matmuls(ph[hb], whc_w, hb)
                    for hb in range(NB):
                        px[hb] = ppool.tile([P, 512], F32, tag="ps", name=f"px{hb}")
                        xc_dr_matmuls(px[hb], wc_w, hb)
                else:
                    for hb in range(NB):
                        pz[hb] = ppool.tile([P, 512], F32, tag="ps", name=f"pz{hb}")
                        gate_matmuls(pz[hb], wz, hb)
                        pr[hb] = ppool.tile([P, 512], F32, tag="ps", name=f"pr{hb}")
                        gate_matmuls(pr[hb], wr, hb)
                        ph[hb] = ppool.tile([P, 512], F32, tag="ps", name=f"ph{hb}")
                        hc_matmuls(ph[hb], whc_w, hb)
                    for hb in range(NB):
                        px[hb] = ppool.tile([P, 512], F32, tag="ps", name=f"px{hb}")
                        xc_matmuls(px[hb], wc_w, hb)

                # --- ACT/DVE: both halves' sigmoids + zh/rh first (they only
                # need the DR psums, available mid-j), then the px-dependent
                # s->tanh->m->o chains, so nothing px-gated blocks the ACT/DVE
                # FIFOs before the fp16 groups finish.
                z_sb = [None, None]
                r_sb = [None, None]
                zh = [None, None]
                rh = [None, None]
                for hb in range(NB):
                    hoff = hb * HB + j * 512  # residual slice of hidden tile j
                    z_sb[hb] = gpool.tile([P, 512], F16, tag="z", name=f"z{hb}")
                    nc.scalar.activation(
                        z_sb[hb][:], pz[hb][:], AF.Sigmoid,
                        bias=bg_sb[:, j : j + 1], scale=INV,
                    )
                    # zh = (z - 1) * h, off the critical path so the
                    # post-tanh chain is only mul + subtract. All-16-bit
                    # operands run the DVE in its 2x mode.
                    zh[hb] = wpool.tile([P, 512], F16, tag="w16", name=f"zh{hb}")
                    nc.vector.scalar_tensor_tensor(
                        zh[hb][:], z_sb[hb][:], 1.0, h16_sb[:, hoff : hoff + 512],
                        ALU.subtract, ALU.mult,
                    )
                    r_sb[hb] = gpool.tile([P, 512], F32, tag="g", name=f"r{hb}")
                    nc.scalar.activation(
                        r_sb[hb][:], pr[hb][:], AF.Sigmoid,
                        bias=bg_sb[:, NJ + j : NJ + j + 1], scale=INV,
                    )
                    # rh = (hc_scaled + b_hc_scaled) * r
                    rh[hb] = wpool.tile([P, 512], F32, tag="w", name=f"rh{hb}")
                    nc.vector.scalar_tensor_tensor(
                        rh[hb][:], ph[hb][:], bhc_sb[:, j : j + 1], r_sb[hb][:],
                        ALU.add, ALU.mult,
                    )

                for hb in range(NB):
                    ooff = j * BL + hb * 512  # output slice
                    last = j == NJ - 1 and hb == NB - 1

                    # s = rh/1024 + px; cand = tanh(s + bc);
                    # out = z*cand - (z-1)*h. The last unit runs in two
                    # 256-wide halves so the post-matmul chain pipelines.
                    def blend(lo, wd, hb=hb, ooff=ooff):
                        s = wpool.tile([P, 512], F32, tag="w")
                        if j == 0:
                            # px is 1024-scaled fp8 here; fold the de-scale
                            # of (px + rh) into the tanh activation
                            nc.vector.tensor_add(
                                s[:, :wd], px[hb][:, lo : lo + wd],
                                rh[hb][:, lo : lo + wd],
                            )
                        else:
                            nc.vector.scalar_tensor_tensor(
                                s[:, :wd], rh[hb][:, lo : lo + wd], INV,
                                px[hb][:, lo : lo + wd], ALU.mult, ALU.add,
                            )
                        cand = wpool.tile([P, 512], F16, tag="w16")
                        nc.scalar.activation(
                            cand[:, :wd], s[:, :wd], AF.Tanh,
                            bias=bc_sb[:, j : j + 1],
                            scale=INV if j == 0 else 1.0,
                        )
                        m = wpool.tile([P, 512], F16, tag="w16")
                        nc.vector.tensor_mul(
                            m[:, :wd], z_sb[hb][:, lo : lo + wd], cand[:, :wd]
                        )
                        o_sb = wpool.tile([P, 512], F16, tag="w16")
                        nc.vector.tensor_sub(o_sb[:, :wd], m[:, :wd], zh[hb][:, lo : lo + wd])
                        nc.scalar.dma_start(
                            outT[:, ooff + lo : ooff + lo + wd], o_sb[:, :wd]
                        )

                    if last:
                        blend(0, 256)
                        blend(256, 256)
                    else:
                        blend(0, 512)

    nc.compile()
    return nc


def _pack_acts(a, dtype, scale=1.0):
    # [BL, H] -> [p, hb*4096 + kc*512 + bb] with scale*a[hb*512+bb, kc*128+p]
    t = (np.asarray(a, np.float32) * scale).reshape(2, 512, KC, P)
    t = np.ascontiguousarray(t.transpose(3, 0, 2, 1).reshape(P, 2 * HB))
    if dtype is E4M3:
        t = np.clip(t, -240.0, 240.0)
    return t.astype(dtype)


def _pack_weights(W_ih, b_ih, W_hh, b_hh, W_c, b_c, W_hc, b_hc):
    Wg_full = np.concatenate([W_ih, W_hh], axis=0) * SW  # [2H, 2H] = [k, o]
    Wg8H = np.ascontiguousarray(
        Wg_full.reshape(16, P, 16, P).transpose(1, 2, 0, 3).reshape(P, 16 * 2048)
    ).astype(E4M3)
    Wc16H = np.ascontiguousarray(
        W_c.reshape(KC, P, NJ, P).transpose(1, 2, 0, 3).reshape(P, NJ * H)
    ).astype(np.float16)
    Whc8H = np.ascontiguousarray(
        (W_hc * SW).reshape(KC, P, NJ, P).transpose(1, 2, 0, 3).reshape(P, NJ * H)
    ).astype(E4M3)
    Wc8j0H = np.ascontiguousarray(
        (W_c[:, :P] * SW).reshape(KC, P, 1, P).transpose(1, 2, 0, 3).reshape(P, H)
    ).astype(E4M3)
    bgH = np.ascontiguousarray((b_ih + b_hh).reshape(16, P).T).astype(np.float32)
    bcH = np.ascontiguousarray(b_c.reshape(NJ, P).T).astype(np.float32)
    bhcsH = np.ascontiguousarray((b_hc / INV).reshape(NJ, P).T).astype(np.float32)
    return Wg8H, Wc16H, Whc8H, Wc8j0H, bgH, bcH, bhcsH


def _make_in_maps(input, hx, W_ih, b_ih, W_hh, b_hh, W_c, b_c, W_hc, b_hc):
    Wg8H, Wc16H, Whc8H, Wc8j0H, bgH, bcH, bhcsH = _pack_weights(
        np.asarray(W_ih, np.float32), np.asarray(b_ih, np.float32),
        np.asarray(W_hh, np.float32), np.asarray(b_hh, np.float32),
        np.asarray(W_c, np.float32), np.asarray(b_c, np.float32),
        np.asarray(W_hc, np.float32), np.asarray(b_hc, np.float32),
    )
    in_maps = []
    for i in range(N_CORES):
        xs = np.asarray(input, np.float32)[i * BL : (i + 1) * BL]
        hs = np.asarray(hx, np.float32)[i * BL : (i + 1) * BL]
        in_maps.append(
            {
                "x8": _pack_acts(xs, E4M3, SA),
                "h8": _pack_acts(hs, E4M3, SA),
                "x16": _pack_acts(xs, np.float16),
                "h16": _pack_acts(hs, np.float16),
                "Wg8": Wg8H,
                "Whc8": Whc8H,
                "Wc16": Wc16H,
                "Wc8j0": Wc8j0H,
                "bg": bgH,
                "bc": bcH,
                "bhcs": bhcsH,
            }
        )
    return in_maps


def kernel(input, hx, W_ih, b_ih, W_hh, b_hh, W_c, b_c, W_hc, b_hc):
    if "nc" not in _CACHE:
        _CACHE["nc"] = _build_program()
    nc = _CACHE["nc"]

    in_maps = _make_in_maps(
        input, hx, W_ih, b_ih, W_hh, b_hh, W_c, b_c, W_hc, b_hc
    )
    res = run_bass_kernel_spmd(nc, in_maps, core_ids=list(range(N_CORES)))
    out = np.empty((B, H), np.float32)
    for i, r in enumerate(res.results):
        o = (
            r["outT"].astype(np.float32)
            .reshape(P, NJ, 2, 512).transpose(2, 3, 1, 0).reshape(BL, H)
        )
        out[i * BL : (i + 1) * BL] = o
    return out


# revision 25
# speedup vs baseline: 1.1925x; 1.1925x over previous
"""GRU-cell-variant kernel for Trainium2, data-parallel over batch on 8 cores.

Reference (per batch row b, hidden size H=1024):
    gates = sigmoid(x @ W_ih + b_ih + h @ W_hh + b_hh)   # [B, 2H]
    z, r  = gates[:, :H], gates[:, H:]
    cand  = tanh(x @ W_c + b_c + r * (h @ W_hc + b_hc))
    out   = (1 - z) * h + z * cand

Design:
  - 8-way batch shard (1024 rows/core), weights replicated. No collectives.
  - Everything on-chip is computed TRANSPOSED: out.T[o, b], so weight tiles
    [k, o] are the stationary operand and host-pre-transposed x.T / h.T the
    moving operand; biases are per-partition (free on the ACT engine).
  - Mixed precision tuned against the 2e-2 rel-err budget (measured on the
    harness inputs): z/r gate matmuls and h@W_hc run in fp8 e4m3 with
    perf_mode=DoubleRow (2 contraction rows per PE cell per cycle);
    x@W_c — the most error-sensitive matmul — stays fp16. Weights are
    pre-scaled by 128 and activations by 8 so e4m3's normal range is used;
    the 1/1024 de-scale folds into the ACT-engine activation scale.
    End-to-end rel err ~1.45e-2 vs 2.05e-2 for all-fp8.
  - Per j, all six DoubleRow gate/hc groups run back-to-back and the two
    fp16 x@W_c groups follow: switching the PE from fp16 back to DoubleRow
    costs ~0.4-0.6us (the DR LDWEIGHTS can't be prefetched during an fp16
    stream), so the order pays that once per j instead of twice.
  - PSUM accumulates fp32; elementwise + residual in fp32 (h residual fp16).
  - ~36 warm-up matmuls on a zeroed SBUF tile run during the DMA preamble so
    the PE HAM clock-gate is already at full rate when real matmuls start.
  - Host packs weights/activations into the exact SBUF layouts so every DMA
    is a dense 2D copy with wide per-partition lines; the cold-start DMAs
    are ordered by PE need time across both HWDGE rings.
"""

import numpy as np
import ml_dtypes

import concourse.bass as bass
import concourse.mybir as mybir
import concourse.tile as tile
from concourse import bacc
from concourse.bass_utils import run_bass_kernel_spmd

N_CORES = 8
B = 8192
H = 1024
BL = B // N_CORES  # batch rows per core
P = 128
KC = H // P  # 8 contraction chunks of 128 per 1024-wide operand
NJ = H // P  # 8 hidden-dim tiles
NB = BL // 512  # 2 moving halves of 512 batch columns
HB = 4096  # fp8 elems per half in [p, hb*4096 + kc*512 + bb] layouts

SW = 128.0  # weight fp8 scale
SA = 8.0  # activation fp8 scale
INV = 1.0 / (SW * SA)  # psum de-scale

N_WARM = 56  # PE warm-up matmuls (N=128) during the DMA preamble

F8 = mybir.dt.float8e4
F16 = mybir.dt.float16
F32 = mybir.dt.float32
AF = mybir.ActivationFunctionType
ALU = mybir.AluOpType
DR = mybir.MatmulPerfMode.DoubleRow

E4M3 = ml_dtypes.float8_e4m3

_CACHE = {}


def _build_program():
    nc = bacc.Bacc(
        "TRN2",
        target_bir_lowering=False,
        debug=False,
        enable_asserts=False,
        num_devices=N_CORES,
    )

    # DRAM inputs, packed on the host (see _pack_* below).
    # x8/h8:  [p, hb*4096 + kc*512 + bb] = 8*a[hb*512+bb, kc*128+p]   (fp8)
    # x16:    same layout, fp16, unscaled (moving operand of x@W_c)
    # h16:    [p, hb*4096 + j*512 + bb] = h[hb*512+bb, j*128+p]       (fp16)
    # Wg8:    [p, t*2048 + kc*128 + jj] = 128*Wg_full[kc*128+p, t*128+jj]
    #          t in [0,16): gate output tile; kc in [0,16): contraction [x;h]
    # Whc8:   [p, j*1024 + kc*128 + jj] = 128*W_hc[kc*128+p, j*128+jj]
    # Wc16:   same transform of W_c, fp16, unscaled
    # bg:     [p, t] = (b_ih+b_hh)[t*128+p]; bc analogous; bhcs = 1024*b_hc
    x8 = nc.dram_tensor("x8", [P, 2 * HB], F8, kind="ExternalInput").ap()
    h8 = nc.dram_tensor("h8", [P, 2 * HB], F8, kind="ExternalInput").ap()
    x16 = nc.dram_tensor("x16", [P, 2 * HB], F16, kind="ExternalInput").ap()
    h16 = nc.dram_tensor("h16", [P, 2 * HB], F16, kind="ExternalInput").ap()
    Wg8 = nc.dram_tensor("Wg8", [P, 16 * 2048], F8, kind="ExternalInput").ap()
    Whc8 = nc.dram_tensor("Whc8", [P, NJ * H], F8, kind="ExternalInput").ap()
    Wc16 = nc.dram_tensor("Wc16", [P, NJ * H], F16, kind="ExternalInput").ap()
    Wc8j0 = nc.dram_tensor("Wc8j0", [P, H], F8, kind="ExternalInput").ap()
    bg = nc.dram_tensor("bg", [P, 16], F32, kind="ExternalInput").ap()
    bc = nc.dram_tensor("bc", [P, NJ], F32, kind="ExternalInput").ap()
    bhcs = nc.dram_tensor("bhcs", [P, NJ], F32, kind="ExternalInput").ap()
    outT = nc.dram_tensor("outT", [P, NJ * BL], F16, kind="ExternalOutput").ap()

    with tile.TileContext(nc) as tc:
        with (
            tc.tile_pool(name="const", bufs=1) as cpool,
            tc.tile_pool(name="wg", bufs=4) as wgpool,
            tc.tile_pool(name="wsm", bufs=4) as wsmpool,
            tc.tile_pool(name="psum", bufs=8, space="PSUM") as ppool,
            tc.tile_pool(name="gates", bufs=8) as gpool,
            tc.tile_pool(name="work", bufs=18) as wpool,
        ):
            bg_sb = cpool.tile([P, 16], F32, tag="bg")
            bc_sb = cpool.tile([P, NJ], F32, tag="bc")
            bhc_sb = cpool.tile([P, NJ], F32, tag="bhc")

            # Resident activations.
            x8_sb = cpool.tile([P, 2 * HB], F8, tag="x8")
            h8_sb = cpool.tile([P, 2 * HB], F8, tag="h8")
            x16_sb = cpool.tile([P, 2 * HB], F16, tag="x16")
            h16_sb = cpool.tile([P, 2 * HB], F16, tag="h16")

            # --- PE warm-up: run dummy matmuls on a zeroed tile while the
            # input DMAs stream, so the HAM clock-gate reaches full rate
            # before the first real matmul issues.
            warm = cpool.tile([P, P], F16, tag="warm")
            nc.vector.memset(warm[:], 0.0)
            pdum = ppool.tile([P, 512], F32, tag="ps")
            for _ in range(N_WARM):
                nc.tensor.matmul(pdum[:, 0:P], lhsT=warm[:], rhs=warm[:])

            def dr2(ap2d):
                # [p, 2*w] slice -> [p, 2, w] DoubleRow AP
                return ap2d.rearrange("p (k m) -> p k m", k=2)

            def gate_matmuls(psum, w_sb, hb, half=None):
                # accumulate over [x;h]: pairs 0-3 read x8, 4-7 read h8.
                # half='x'/'h' emits only that part of the accumulation
                # group (cold start: x-only work runs while h still streams).
                cs = range(KC) if half is None else (
                    range(KC // 2) if half == "x" else range(KC // 2, KC)
                )
                for c in cs:
                    src = x8_sb if c < 4 else h8_sb
                    o = hb * HB + ((2 * c) % KC) * 512
                    nc.tensor.matmul(
                        psum[:],
                        lhsT=dr2(w_sb[:, 2 * c * P : (2 * c + 2) * P]),
                        rhs=dr2(src[:, o : o + 1024]),
                        start=(c == 0),
                        stop=(c == KC - 1),
                        perf_mode=DR,
                    )

            def hc_matmuls(psum, w_sb, hb):
                for c in range(KC // 2):
                    o = hb * HB + 2 * c * 512
                    nc.tensor.matmul(
                        psum[:],
                        lhsT=dr2(w_sb[:, 2 * c * P : (2 * c + 2) * P]),
                        rhs=dr2(h8_sb[:, o : o + 1024]),
                        start=(c == 0),
                        stop=(c == KC // 2 - 1),
                        perf_mode=DR,
                    )

            def xc_dr_matmuls(psum, w_sb, hb):
                # j0 cold-start path: x@W_c in fp8 DoubleRow off resident x8,
                # so j0 needs neither x16 nor an fp16<->DR mode switch.
                for c in range(KC // 2):
                    o = hb * HB + 2 * c * 512
                    nc.tensor.matmul(
                        psum[:],
                        lhsT=dr2(w_sb[:, 2 * c * P : (2 * c + 2) * P]),
                        rhs=dr2(x8_sb[:, o : o + 1024]),
                        start=(c == 0),
                        stop=(c == KC // 2 - 1),
                        perf_mode=DR,
                    )

            def xc_matmuls(psum, w_sb, hb):
                for kc in range(KC):
                    o = hb * HB + kc * 512
                    nc.tensor.matmul(
                        psum[:],
                        lhsT=w_sb[:, kc * P : (kc + 1) * P],
                        rhs=x16_sb[:, o : o + 512],
                        start=(kc == 0),
                        stop=(kc == KC - 1),
                    )

            for j in range(NJ):
                wz = wgpool.tile([P, 2048], F8, tag="wg")
                wr = wgpool.tile([P, 2048], F8, tag="wg")
                whc_w = wsmpool.tile([P, H], F8, tag="whc")
                if j == 0:
                    wc_w = wsmpool.tile([P, H], F8, tag="whc", name="wc8_0")
                else:
                    wc_w = wsmpool.tile([P, H], F16, tag="wc")
                if j == 0:
                    # Cold-start feed, ordered by PE need time. sync ring:
                    # gate activations (both halves). ACT ring: j0 weights,
                    # constants, residual h16 j0.
                    HB2 = HB // 2
                    nc.sync.dma_start(x8_sb[:, 0:HB2], x8[:, 0:HB2])
                    nc.sync.dma_start(x8_sb[:, HB2:HB], x8[:, HB2:HB])
                    nc.sync.dma_start(h8_sb[:, 0:HB2], h8[:, 0:HB2])
                    nc.sync.dma_start(h8_sb[:, HB2:HB], h8[:, HB2:HB])
                    nc.sync.dma_start(x8_sb[:, HB : 2 * HB], x8[:, HB : 2 * HB])
                    nc.sync.dma_start(h8_sb[:, HB : 2 * HB], h8[:, HB : 2 * HB])
                    nc.scalar.dma_start(wz[:, 0:1024], Wg8[:, 0:1024])
                    nc.scalar.dma_start(wz[:, 1024:2048], Wg8[:, 1024:2048])
                    nc.scalar.dma_start(bg_sb[:], bg[:])
                    nc.scalar.dma_start(wr[:, 0:1024], Wg8[:, NJ * 2048 : NJ * 2048 + 1024])
                    nc.scalar.dma_start(wr[:, 1024:2048], Wg8[:, NJ * 2048 + 1024 : NJ * 2048 + 2048])
                    nc.scalar.dma_start(whc_w[:], Whc8[:, 0:H])
                    nc.scalar.dma_start(bc_sb[:], bc[:])
                    nc.scalar.dma_start(bhc_sb[:], bhcs[:])
                    nc.scalar.dma_start(wc_w[:], Wc8j0[:])
                    nc.scalar.dma_start(h16_sb[:, 0:512], h16[:, 0:512])
                    nc.scalar.dma_start(h16_sb[:, HB : HB + 512], h16[:, HB : HB + 512])
                elif j == 1:
                    # split j=1 weights across the two rings
                    nc.sync.dma_start(wz[:], Wg8[:, 1 * 2048 : 2 * 2048])
                    nc.scalar.dma_start(wr[:], Wg8[:, (NJ + 1) * 2048 : (NJ + 2) * 2048])
                    nc.sync.dma_start(whc_w[:], Whc8[:, H : 2 * H])
                    nc.scalar.dma_start(wc_w[:], Wc16[:, H : 2 * H])
                else:
                    nc.sync.dma_start(wz[:], Wg8[:, j * 2048 : (j + 1) * 2048])
                    nc.sync.dma_start(wr[:], Wg8[:, (NJ + j) * 2048 : (NJ + j + 1) * 2048])
                    nc.sync.dma_start(whc_w[:], Whc8[:, j * H : (j + 1) * H])
                    nc.sync.dma_start(wc_w[:], Wc16[:, j * H : (j + 1) * H])
                if j > 0:
                    # residual-path h (fp16) rides the ACT ring, per (j, half)
                    for hb in range(2):
                        o = hb * HB + j * 512
                        nc.scalar.dma_start(h16_sb[:, o : o + 512], h16[:, o : o + 512])
                if j == 1:
                    # x16 is first consumed by j1's px groups — load it after
                    # j1's weights so it doesn't block them in the queues
                    HB2 = HB // 2
                    nc.sync.dma_start(x16_sb[:, 0:HB2], x16[:, 0:HB2])
                    nc.scalar.dma_start(x16_sb[:, HB2:HB], x16[:, HB2:HB])
                    nc.sync.dma_start(x16_sb[:, HB : HB + HB2], x16[:, HB : HB + HB2])
                    nc.scalar.dma_start(x16_sb[:, HB + HB2 : 2 * HB], x16[:, HB + HB2 : 2 * HB])

                # --- PE: all DoubleRow groups back-to-back, then the two
                # fp16 x@W_c groups (one DR<-fp16 mode switch per j).
                pz = [None, None]
                pr = [None, None]
                ph = [None, None]
                px = [None, None]
                if j == 0:
                    # Cold start: per half, emit the gate x-halves first and
                    # bridge the wait for the h8 stream with dummy matmuls
                    # (separate PSUM bank) so the PE never idles long enough
                    # for the HAM clock-gate to re-throttle.
                    for hb in range(NB):
                        pz[hb] = ppool.tile([P, 512], F32, tag="ps", name=f"pz{hb}")
                        gate_matmuls(pz[hb], wz, hb, half="x")
                        pr[hb] = ppool.tile([P, 512], F32, tag="ps", name=f"pr{hb}")
                        gate_matmuls(pr[hb], wr, hb, half="x")
                        for _ in range(26 if hb == 0 else 15):
                            nc.tensor.matmul(pdum[:, 0:P], lhsT=warm[:], rhs=warm[:])
                        gate_matmuls(pz[hb], wz, hb, half="h")
                        gate_matmuls(pr[hb], wr, hb, half="h")
                        ph[hb] = ppool.tile([P, 512], F32, tag="ps", name=f"ph{hb}")
                        hc_matmuls(ph[hb], whc_w, hb)
                    for hb in range(NB):
                        px[hb] = ppool.tile([P, 512], F32, tag="ps", name=f"px{hb}")
                        xc_dr_matmuls(px[hb], wc_w, hb)
                else:
                    for hb in range(NB):
                        pz[hb] = ppool.tile([P, 512], F32, tag="ps", name=f"pz{hb}")
                        gate_matmuls(pz[hb], wz, hb)
                        pr[hb] = ppool.tile([P, 512], F32, tag="ps", name=f"pr{hb}")
                        gate_matmuls(pr[hb], wr, hb)
                        ph[hb] = ppool.tile([P, 512], F32, tag="ps", name=f"ph{hb}")
                        hc_matmuls(ph[hb], whc_w, hb)
                    pxs = [None, None]
                    for hb in range(NB):
                        if j == NJ - 1 and hb == NB - 1:
                            # split the very last fp16 group in two 256-col
                            # halves: the first half's blend chain runs while
                            # the second half's matmuls still stream
                            for i, lo in enumerate((0, 256)):
                                pxs[i] = ppool.tile(
                                    [P, 512], F32, tag="ps", name=f"pxs{i}"
                                )
                                for kc in range(KC):
                                    o = hb * HB + kc * 512 + lo
                                    nc.tensor.matmul(
                                        pxs[i][:, 0:256],
                                        lhsT=wc_w[:, kc * P : (kc + 1) * P],
                                        rhs=x16_sb[:, o : o + 256],
                                        start=(kc == 0),
                                        stop=(kc == KC - 1),
                                    )
                        else:
                            px[hb] = ppool.tile([P, 512], F32, tag="ps", name=f"px{hb}")
                            xc_matmuls(px[hb], wc_w, hb)

                # --- ACT/DVE: both halves' sigmoids + zh/rh first (they only
                # need the DR psums, available mid-j), then the px-dependent
                # s->tanh->m->o chains, so nothing px-gated blocks the ACT/DVE
                # FIFOs before the fp16 groups finish.
                z_sb = [None, None]
                r_sb = [None, None]
                zh = [None, None]
                rh = [None, None]
                for hb in range(NB):
                    hoff = hb * HB + j * 512  # residual slice of hidden tile j
                    z_sb[hb] = gpool.tile([P, 512], F16, tag="z", name=f"z{hb}")
                    nc.scalar.activation(
                        z_sb[hb][:], pz[hb][:], AF.Sigmoid,
                        bias=bg_sb[:, j : j + 1], scale=INV,
                    )
                    # zh = (z - 1) * h, off the critical path so the
                    # post-tanh chain is only mul + subtract. All-16-bit
                    # operands run the DVE in its 2x mode.
                    zh[hb] = wpool.tile([P, 512], F16, tag="w16", name=f"zh{hb}")
                    nc.vector.scalar_tensor_tensor(
                        zh[hb][:], z_sb[hb][:], 1.0, h16_sb[:, hoff : hoff + 512],
                        ALU.subtract, ALU.mult,
                    )
                    r_sb[hb] = gpool.tile([P, 512], F32, tag="g", name=f"r{hb}")
                    nc.scalar.activation(
                        r_sb[hb][:], pr[hb][:], AF.Sigmoid,
                        bias=bg_sb[:, NJ + j : NJ + j + 1], scale=INV,
                    )
                    # rh = (hc_scaled + b_hc_scaled) * r
                    rh[hb] = wpool.tile([P, 512], F32, tag="w", name=f"rh{hb}")
                    nc.vector.scalar_tensor_tensor(
                        rh[hb][:], ph[hb][:], bhc_sb[:, j : j + 1], r_sb[hb][:],
                        ALU.add, ALU.mult,
                    )

                for hb in range(NB):
                    ooff = j * BL + hb * 512  # output slice
                    last = j == NJ - 1 and hb == NB - 1

                    # s = rh/1024 + px; cand = tanh(s + bc);
                    # out = z*cand - (z-1)*h. The last unit runs in two
                    # 256-wide halves so the post-matmul chain pipelines.
                    def blend(lo, wd, hb=hb, ooff=ooff, pap=None, pofs=None):
                        if pap is None:
                            pap, pofs = px[hb], lo
                        s = wpool.tile([P, 512], F32, tag="w")
                        if j == 0:
                            # px is 1024-scaled fp8 here; fold the de-scale
                            # of (px + rh) into the tanh activation
                            nc.vector.tensor_add(
                                s[:, :wd], pap[:, pofs : pofs + wd],
                                rh[hb][:, lo : lo + wd],
                            )
                        else:
                            nc.vector.scalar_tensor_tensor(
                                s[:, :wd], rh[hb][:, lo : lo + wd], INV,
                                pap[:, pofs : pofs + wd], ALU.mult, ALU.add,
                            )
                        cand = wpool.tile([P, 512], F16, tag="w16")
                        nc.scalar.activation(
                            cand[:, :wd], s[:, :wd], AF.Tanh,
                            bias=bc_sb[:, j : j + 1],
                            scale=INV if j == 0 else 1.0,
                        )
                        m = wpool.tile([P, 512], F16, tag="w16")
                        nc.vector.tensor_mul(
                            m[:, :wd], z_sb[hb][:, lo : lo + wd], cand[:, :wd]
                        )
                        o_sb = wpool.tile([P, 512], F16, tag="w16")
                        nc.vector.tensor_sub(o_sb[:, :wd], m[:, :wd], zh[hb][:, lo : lo + wd])
                        nc.scalar.dma_start(
                            outT[:, ooff + lo : ooff + lo + wd], o_sb[:, :wd]
                        )

                    if last:
                        blend(0, 256, pap=pxs[0], pofs=0)
                        blend(256, 256, pap=pxs[1], pofs=0)
                    else:
                        blend(0, 512)

    nc.compile()
    return nc


def _pack_acts(a, dtype, scale=1.0):
    # [BL, H] -> [p, hb*4096 + kc*512 + bb] with scale*a[hb*512+bb, kc*128+p]
    t = (np.asarray(a, np.float32) * scale).reshape(2, 512, KC, P)
    t = np.ascontiguousarray(t.transpose(3, 0, 2, 1).reshape(P, 2 * HB))
    if dtype is E4M3:
        t = np.clip(t, -240.0, 240.0)
    return t.astype(dtype)


def _pack_weights(W_ih, b_ih, W_hh, b_hh, W_c, b_c, W_hc, b_hc):
    Wg_full = np.concatenate([W_ih, W_hh], axis=0) * SW  # [2H, 2H] = [k, o]
    Wg8H = np.ascontiguousarray(
        Wg_full.reshape(16, P, 16, P).transpose(1, 2, 0, 3).reshape(P, 16 * 2048)
    ).astype(E4M3)
    Wc16H = np.ascontiguousarray(
        W_c.reshape(KC, P, NJ, P).transpose(1, 2, 0, 3).reshape(P, NJ * H)
    ).astype(np.float16)
    Whc8H = np.ascontiguousarray(
        (W_hc * SW).reshape(KC, P, NJ, P).transpose(1, 2, 0, 3).reshape(P, NJ * H)
    ).astype(E4M3)
    Wc8j0H = np.ascontiguousarray(
        (W_c[:, :P] * SW).reshape(KC, P, 1, P).transpose(1, 2, 0, 3).reshape(P, H)
    ).astype(E4M3)
    bgH = np.ascontiguousarray((b_ih + b_hh).reshape(16, P).T).astype(np.float32)
    bcH = np.ascontiguousarray(b_c.reshape(NJ, P).T).astype(np.float32)
    bhcsH = np.ascontiguousarray((b_hc / INV).reshape(NJ, P).T).astype(np.float32)
    return Wg8H, Wc16H, Whc8H, Wc8j0H, bgH, bcH, bhcsH


def _make_in_maps(input, hx, W_ih, b_ih, W_hh, b_hh, W_c, b_c, W_hc, b_hc):
    Wg8H, Wc16H, Whc8H, Wc8j0H, bgH, bcH, bhcsH = _pack_weights(
        np.asarray(W_ih, np.float32), np.asarray(b_ih, np.float32),
        np.asarray(W_hh, np.float32), np.asarray(b_hh, np.float32),
        np.asarray(W_c, np.float32), np.asarray(b_c, np.float32),
        np.asarray(W_hc, np.float32), np.asarray(b_hc, np.float32),
    )
    in_maps = []
    for i in range(N_CORES):
        xs = np.asarray(input, np.float32)[i * BL : (i + 1) * BL]
        hs = np.asarray(hx, np.float32)[i * BL : (i + 1) * BL]
        in_maps.append(
            {
                "x8": _pack_acts(xs, E4M3, SA),
                "h8": _pack_acts(hs, E4M3, SA),
                "x16": _pack_acts(xs, np.float16),
                "h16": _pack_acts(hs, np.float16),
                "Wg8": Wg8H,
                "Whc8": Whc8H,
                "Wc16": Wc16H,
                "Wc8j0": Wc8j0H,
                "bg": bgH,
                "bc": bcH,
                "bhcs": bhcsH,
            }
        )
    return in_maps


def kernel(input, hx, W_ih, b_ih, W_hh, b_hh, W_c, b_c, W_hc, b_hc):
    if "nc" not in _CACHE:
        _CACHE["nc"] = _build_program()
    nc = _CACHE["nc"]

    in_maps = _make_in_maps(
        input, hx, W_ih, b_ih, W_hh, b_hh, W_c, b_c, W_hc, b_hc
    )
    res = run_bass_kernel_spmd(nc, in_maps, core_ids=list(range(N_CORES)))
    out = np.empty((B, H), np.float32)
    for i, r in enumerate(res.results):
        o = (
            r["outT"].astype(np.float32)
            .reshape(P, NJ, 2, 512).transpose(2, 3, 1, 0).reshape(BL, H)
        )
        out[i * BL : (i + 1) * BL] = o
    return out


# revision 26
# speedup vs baseline: 1.2182x; 1.0215x over previous
"""GRU-cell-variant kernel for Trainium2, data-parallel over batch on 8 cores.

Reference (per batch row b, hidden size H=1024):
    gates = sigmoid(x @ W_ih + b_ih + h @ W_hh + b_hh)   # [B, 2H]
    z, r  = gates[:, :H], gates[:, H:]
    cand  = tanh(x @ W_c + b_c + r * (h @ W_hc + b_hc))
    out   = (1 - z) * h + z * cand

Design:
  - 8-way batch shard (1024 rows/core), weights replicated. No collectives.
  - Everything on-chip is computed TRANSPOSED: out.T[o, b], so weight tiles
    [k, o] are the stationary operand and host-pre-transposed x.T / h.T the
    moving operand; biases are per-partition (free on the ACT engine).
  - Mixed precision tuned against the 2e-2 rel-err budget (measured on the
    harness inputs): z/r gate matmuls and h@W_hc run in fp8 e4m3 with
    perf_mode=DoubleRow (2 contraction rows per PE cell per cycle);
    x@W_c — the most error-sensitive matmul — stays fp16. Weights are
    pre-scaled by 128 and activations by 8 so e4m3's normal range is used;
    the 1/1024 de-scale folds into the ACT-engine activation scale.
    End-to-end rel err ~1.45e-2 vs 2.05e-2 for all-fp8.
  - Per j, all six DoubleRow gate/hc groups run back-to-back and the two
    fp16 x@W_c groups follow: switching the PE from fp16 back to DoubleRow
    costs ~0.4-0.6us (the DR LDWEIGHTS can't be prefetched during an fp16
    stream), so the order pays that once per j instead of twice.
  - PSUM accumulates fp32; elementwise + residual in fp32 (h residual fp16).
  - ~36 warm-up matmuls on a zeroed SBUF tile run during the DMA preamble so
    the PE HAM clock-gate is already at full rate when real matmuls start.
  - Host packs weights/activations into the exact SBUF layouts so every DMA
    is a dense 2D copy with wide per-partition lines; the cold-start DMAs
    are ordered by PE need time across both HWDGE rings.
"""

import numpy as np
import ml_dtypes

import concourse.bass as bass
import concourse.mybir as mybir
import concourse.tile as tile
from concourse import bacc
from concourse.bass_utils import run_bass_kernel_spmd

N_CORES = 8
B = 8192
H = 1024
BL = B // N_CORES  # batch rows per core
P = 128
KC = H // P  # 8 contraction chunks of 128 per 1024-wide operand
NJ = H // P  # 8 hidden-dim tiles
NB = BL // 512  # 2 moving halves of 512 batch columns
HB = 4096  # fp8 elems per half in [p, hb*4096 + kc*512 + bb] layouts

SW = 128.0  # weight fp8 scale
SA = 8.0  # activation fp8 scale
INV = 1.0 / (SW * SA)  # psum de-scale

N_WARM = 56  # PE warm-up matmuls (N=128) during the DMA preamble

F8 = mybir.dt.float8e4
F16 = mybir.dt.float16
F32 = mybir.dt.float32
AF = mybir.ActivationFunctionType
ALU = mybir.AluOpType
DR = mybir.MatmulPerfMode.DoubleRow

E4M3 = ml_dtypes.float8_e4m3

_CACHE = {}


def _build_program():
    nc = bacc.Bacc(
        "TRN2",
        target_bir_lowering=False,
        debug=False,
        enable_asserts=False,
        num_devices=N_CORES,
    )

    # DRAM inputs, packed on the host (see _pack_* below).
    # x8/h8:  [p, hb*4096 + kc*512 + bb] = 8*a[hb*512+bb, kc*128+p]   (fp8)
    # x16:    same layout, fp16, unscaled (moving operand of x@W_c)
    # h16:    [p, hb*4096 + j*512 + bb] = h[hb*512+bb, j*128+p]       (fp16)
    # Wg8:    [p, t*2048 + kc*128 + jj] = 128*Wg_full[kc*128+p, t*128+jj]
    #          t in [0,16): gate output tile; kc in [0,16): contraction [x;h]
    # Whc8:   [p, j*1024 + kc*128 + jj] = 128*W_hc[kc*128+p, j*128+jj]
    # Wc16:   same transform of W_c, fp16, unscaled
    # bg:     [p, t] = (b_ih+b_hh)[t*128+p]; bc analogous; bhcs = 1024*b_hc
    x8 = nc.dram_tensor("x8", [P, 2 * HB], F8, kind="ExternalInput").ap()
    h8 = nc.dram_tensor("h8", [P, 2 * HB], F8, kind="ExternalInput").ap()
    x16 = nc.dram_tensor("x16", [P, 2 * HB], F16, kind="ExternalInput").ap()
    h16 = nc.dram_tensor("h16", [P, 2 * HB], F16, kind="ExternalInput").ap()
    Wg8 = nc.dram_tensor("Wg8", [P, 16 * 2048], F8, kind="ExternalInput").ap()
    Whc8 = nc.dram_tensor("Whc8", [P, NJ * H], F8, kind="ExternalInput").ap()
    Wc16 = nc.dram_tensor("Wc16", [P, NJ * H], F16, kind="ExternalInput").ap()
    Wc8lo = nc.dram_tensor("Wc8lo", [P, 2 * H], F8, kind="ExternalInput").ap()
    bg = nc.dram_tensor("bg", [P, 16], F32, kind="ExternalInput").ap()
    bc = nc.dram_tensor("bc", [P, NJ], F32, kind="ExternalInput").ap()
    bhcs = nc.dram_tensor("bhcs", [P, NJ], F32, kind="ExternalInput").ap()
    outT = nc.dram_tensor("outT", [P, NJ * BL], F16, kind="ExternalOutput").ap()

    with tile.TileContext(nc) as tc:
        with (
            tc.tile_pool(name="const", bufs=1) as cpool,
            tc.tile_pool(name="wg", bufs=4) as wgpool,
            tc.tile_pool(name="wsm", bufs=4) as wsmpool,
            tc.tile_pool(name="psum", bufs=8, space="PSUM") as ppool,
            tc.tile_pool(name="gates", bufs=8) as gpool,
            tc.tile_pool(name="work", bufs=18) as wpool,
        ):
            bg_sb = cpool.tile([P, 16], F32, tag="bg")
            bc_sb = cpool.tile([P, NJ], F32, tag="bc")
            bhc_sb = cpool.tile([P, NJ], F32, tag="bhc")

            # Resident activations.
            x8_sb = cpool.tile([P, 2 * HB], F8, tag="x8")
            h8_sb = cpool.tile([P, 2 * HB], F8, tag="h8")
            x16_sb = cpool.tile([P, 2 * HB], F16, tag="x16")
            h16_sb = cpool.tile([P, 2 * HB], F16, tag="h16")

            # --- PE warm-up: run dummy matmuls on a zeroed tile while the
            # input DMAs stream, so the HAM clock-gate reaches full rate
            # before the first real matmul issues.
            warm = cpool.tile([P, P], F16, tag="warm")
            nc.vector.memset(warm[:], 0.0)
            pdum = ppool.tile([P, 512], F32, tag="ps")
            for _ in range(N_WARM):
                nc.tensor.matmul(pdum[:, 0:P], lhsT=warm[:], rhs=warm[:])

            def dr2(ap2d):
                # [p, 2*w] slice -> [p, 2, w] DoubleRow AP
                return ap2d.rearrange("p (k m) -> p k m", k=2)

            def gate_matmuls(psum, w_sb, hb, half=None):
                # accumulate over [x;h]: pairs 0-3 read x8, 4-7 read h8.
                # half='x'/'h' emits only that part of the accumulation
                # group (cold start: x-only work runs while h still streams).
                cs = range(KC) if half is None else (
                    range(KC // 2) if half == "x" else range(KC // 2, KC)
                )
                for c in cs:
                    src = x8_sb if c < 4 else h8_sb
                    o = hb * HB + ((2 * c) % KC) * 512
                    nc.tensor.matmul(
                        psum[:],
                        lhsT=dr2(w_sb[:, 2 * c * P : (2 * c + 2) * P]),
                        rhs=dr2(src[:, o : o + 1024]),
                        start=(c == 0),
                        stop=(c == KC - 1),
                        perf_mode=DR,
                    )

            def hc_matmuls(psum, w_sb, hb):
                for c in range(KC // 2):
                    o = hb * HB + 2 * c * 512
                    nc.tensor.matmul(
                        psum[:],
                        lhsT=dr2(w_sb[:, 2 * c * P : (2 * c + 2) * P]),
                        rhs=dr2(h8_sb[:, o : o + 1024]),
                        start=(c == 0),
                        stop=(c == KC // 2 - 1),
                        perf_mode=DR,
                    )

            def xc_dr_matmuls(psum, w_sb, hb):
                # j0 cold-start path: x@W_c in fp8 DoubleRow off resident x8,
                # so j0 needs neither x16 nor an fp16<->DR mode switch.
                for c in range(KC // 2):
                    o = hb * HB + 2 * c * 512
                    nc.tensor.matmul(
                        psum[:],
                        lhsT=dr2(w_sb[:, 2 * c * P : (2 * c + 2) * P]),
                        rhs=dr2(x8_sb[:, o : o + 1024]),
                        start=(c == 0),
                        stop=(c == KC // 2 - 1),
                        perf_mode=DR,
                    )

            def xc_matmuls(psum, w_sb, hb):
                for kc in range(KC):
                    o = hb * HB + kc * 512
                    nc.tensor.matmul(
                        psum[:],
                        lhsT=w_sb[:, kc * P : (kc + 1) * P],
                        rhs=x16_sb[:, o : o + 512],
                        start=(kc == 0),
                        stop=(kc == KC - 1),
                    )

            for j in range(NJ):
                wz = wgpool.tile([P, 2048], F8, tag="wg")
                wr = wgpool.tile([P, 2048], F8, tag="wg")
                whc_w = wsmpool.tile([P, H], F8, tag="whc")
                if j <= 1:
                    wc_w = wsmpool.tile([P, H], F8, tag="whc", name=f"wc8_{j}")
                else:
                    wc_w = wsmpool.tile([P, H], F16, tag="wc")
                if j == 0:
                    # Cold-start feed, ordered by PE need time. sync ring:
                    # gate activations (both halves). ACT ring: j0 weights,
                    # constants, residual h16 j0.
                    HB2 = HB // 2
                    nc.sync.dma_start(x8_sb[:, 0:HB2], x8[:, 0:HB2])
                    nc.sync.dma_start(x8_sb[:, HB2:HB], x8[:, HB2:HB])
                    nc.sync.dma_start(h8_sb[:, 0:HB2], h8[:, 0:HB2])
                    nc.sync.dma_start(h8_sb[:, HB2:HB], h8[:, HB2:HB])
                    nc.sync.dma_start(x8_sb[:, HB : 2 * HB], x8[:, HB : 2 * HB])
                    nc.sync.dma_start(h8_sb[:, HB : 2 * HB], h8[:, HB : 2 * HB])
                    nc.scalar.dma_start(wz[:, 0:1024], Wg8[:, 0:1024])
                    nc.scalar.dma_start(wz[:, 1024:2048], Wg8[:, 1024:2048])
                    nc.scalar.dma_start(bg_sb[:], bg[:])
                    nc.scalar.dma_start(wr[:, 0:1024], Wg8[:, NJ * 2048 : NJ * 2048 + 1024])
                    nc.scalar.dma_start(wr[:, 1024:2048], Wg8[:, NJ * 2048 + 1024 : NJ * 2048 + 2048])
                    nc.scalar.dma_start(whc_w[:], Whc8[:, 0:H])
                    nc.scalar.dma_start(bc_sb[:], bc[:])
                    nc.scalar.dma_start(bhc_sb[:], bhcs[:])
                    nc.scalar.dma_start(wc_w[:], Wc8lo[:, 0:H])
                    nc.scalar.dma_start(h16_sb[:, 0:512], h16[:, 0:512])
                    nc.scalar.dma_start(h16_sb[:, HB : HB + 512], h16[:, HB : HB + 512])
                elif j == 1:
                    # split j=1 weights across the two rings
                    nc.sync.dma_start(wz[:], Wg8[:, 1 * 2048 : 2 * 2048])
                    nc.scalar.dma_start(wr[:], Wg8[:, (NJ + 1) * 2048 : (NJ + 2) * 2048])
                    nc.sync.dma_start(whc_w[:], Whc8[:, H : 2 * H])
                    nc.scalar.dma_start(wc_w[:], Wc8lo[:, H : 2 * H])
                else:
                    nc.sync.dma_start(wz[:], Wg8[:, j * 2048 : (j + 1) * 2048])
                    nc.sync.dma_start(wr[:], Wg8[:, (NJ + j) * 2048 : (NJ + j + 1) * 2048])
                    nc.sync.dma_start(whc_w[:], Whc8[:, j * H : (j + 1) * H])
                    nc.sync.dma_start(wc_w[:], Wc16[:, j * H : (j + 1) * H])
                if j > 0:
                    # residual-path h (fp16) rides the ACT ring, per (j, half)
                    for hb in range(2):
                        o = hb * HB + j * 512
                        nc.scalar.dma_start(h16_sb[:, o : o + 512], h16[:, o : o + 512])
                if j == 1:
                    # x16 is first consumed by j1's px groups — load it after
                    # j1's weights so it doesn't block them in the queues
                    HB2 = HB // 2
                    nc.sync.dma_start(x16_sb[:, 0:HB2], x16[:, 0:HB2])
                    nc.scalar.dma_start(x16_sb[:, HB2:HB], x16[:, HB2:HB])
                    nc.sync.dma_start(x16_sb[:, HB : HB + HB2], x16[:, HB : HB + HB2])
                    nc.scalar.dma_start(x16_sb[:, HB + HB2 : 2 * HB], x16[:, HB + HB2 : 2 * HB])

                # --- PE: all DoubleRow groups back-to-back, then the two
                # fp16 x@W_c groups (one DR<-fp16 mode switch per j).
                pz = [None, None]
                pr = [None, None]
                ph = [None, None]
                px = [None, None]
                if j == 0:
                    # Cold start: per half, emit the gate x-halves first and
                    # bridge the wait for the h8 stream with dummy matmuls
                    # (separate PSUM bank) so the PE never idles long enough
                    # for the HAM clock-gate to re-throttle.
                    for hb in range(NB):
                        pz[hb] = ppool.tile([P, 512], F32, tag="ps", name=f"pz{hb}")
                        gate_matmuls(pz[hb], wz, hb, half="x")
                        pr[hb] = ppool.tile([P, 512], F32, tag="ps", name=f"pr{hb}")
                        gate_matmuls(pr[hb], wr, hb, half="x")
                        for _ in range(26 if hb == 0 else 15):
                            nc.tensor.matmul(pdum[:, 0:P], lhsT=warm[:], rhs=warm[:])
                        gate_matmuls(pz[hb], wz, hb, half="h")
                        gate_matmuls(pr[hb], wr, hb, half="h")
                        ph[hb] = ppool.tile([P, 512], F32, tag="ps", name=f"ph{hb}")
                        hc_matmuls(ph[hb], whc_w, hb)
                    for hb in range(NB):
                        px[hb] = ppool.tile([P, 512], F32, tag="ps", name=f"px{hb}")
                        xc_dr_matmuls(px[hb], wc_w, hb)
                else:
                    for hb in range(NB):
                        pz[hb] = ppool.tile([P, 512], F32, tag="ps", name=f"pz{hb}")
                        gate_matmuls(pz[hb], wz, hb)
                        pr[hb] = ppool.tile([P, 512], F32, tag="ps", name=f"pr{hb}")
                        gate_matmuls(pr[hb], wr, hb)
                        ph[hb] = ppool.tile([P, 512], F32, tag="ps", name=f"ph{hb}")
                        hc_matmuls(ph[hb], whc_w, hb)
                    pxs = [None, None]
                    for hb in range(NB):
                        if j == NJ - 1 and hb == NB - 1:
                            # split the very last fp16 group in two 256-col
                            # halves: the first half's blend chain runs while
                            # the second half's matmuls still stream
                            for i, lo in enumerate((0, 256)):
                                pxs[i] = ppool.tile(
                                    [P, 512], F32, tag="ps", name=f"pxs{i}"
                                )
                                for kc in range(KC):
                                    o = hb * HB + kc * 512 + lo
                                    nc.tensor.matmul(
                                        pxs[i][:, 0:256],
                                        lhsT=wc_w[:, kc * P : (kc + 1) * P],
                                        rhs=x16_sb[:, o : o + 256],
                                        start=(kc == 0),
                                        stop=(kc == KC - 1),
                                    )
                        else:
                            px[hb] = ppool.tile([P, 512], F32, tag="ps", name=f"px{hb}")
                            if j == 1:
                                xc_dr_matmuls(px[hb], wc_w, hb)
                            else:
                                xc_matmuls(px[hb], wc_w, hb)

                # --- ACT/DVE: both halves' sigmoids + zh/rh first (they only
                # need the DR psums, available mid-j), then the px-dependent
                # s->tanh->m->o chains, so nothing px-gated blocks the ACT/DVE
                # FIFOs before the fp16 groups finish.
                z_sb = [None, None]
                r_sb = [None, None]
                zh = [None, None]
                rh = [None, None]
                for hb in range(NB):
                    hoff = hb * HB + j * 512  # residual slice of hidden tile j
                    z_sb[hb] = gpool.tile([P, 512], F16, tag="z", name=f"z{hb}")
                    nc.scalar.activation(
                        z_sb[hb][:], pz[hb][:], AF.Sigmoid,
                        bias=bg_sb[:, j : j + 1], scale=INV,
                    )
                    # zh = (z - 1) * h, off the critical path so the
                    # post-tanh chain is only mul + subtract. All-16-bit
                    # operands run the DVE in its 2x mode.
                    zh[hb] = wpool.tile([P, 512], F16, tag="w16", name=f"zh{hb}")
                    nc.vector.scalar_tensor_tensor(
                        zh[hb][:], z_sb[hb][:], 1.0, h16_sb[:, hoff : hoff + 512],
                        ALU.subtract, ALU.mult,
                    )
                    r_sb[hb] = gpool.tile([P, 512], F32, tag="g", name=f"r{hb}")
                    nc.scalar.activation(
                        r_sb[hb][:], pr[hb][:], AF.Sigmoid,
                        bias=bg_sb[:, NJ + j : NJ + j + 1], scale=INV,
                    )
                    # rh = (hc_scaled + b_hc_scaled) * r
                    rh[hb] = wpool.tile([P, 512], F32, tag="w", name=f"rh{hb}")
                    nc.vector.scalar_tensor_tensor(
                        rh[hb][:], ph[hb][:], bhc_sb[:, j : j + 1], r_sb[hb][:],
                        ALU.add, ALU.mult,
                    )

                for hb in range(NB):
                    ooff = j * BL + hb * 512  # output slice
                    last = j == NJ - 1 and hb == NB - 1

                    # s = rh/1024 + px; cand = tanh(s + bc);
                    # out = z*cand - (z-1)*h. The last unit runs in two
                    # 256-wide halves so the post-matmul chain pipelines.
                    def blend(lo, wd, hb=hb, ooff=ooff, pap=None, pofs=None):
                        if pap is None:
                            pap, pofs = px[hb], lo
                        s = wpool.tile([P, 512], F32, tag="w")
                        if j <= 1:
                            # px is 1024-scaled fp8 here; fold the de-scale
                            # of (px + rh) into the tanh activation
                            nc.vector.tensor_add(
                                s[:, :wd], pap[:, pofs : pofs + wd],
                                rh[hb][:, lo : lo + wd],
                            )
                        else:
                            nc.vector.scalar_tensor_tensor(
                                s[:, :wd], rh[hb][:, lo : lo + wd], INV,
                                pap[:, pofs : pofs + wd], ALU.mult, ALU.add,
                            )
                        cand = wpool.tile([P, 512], F16, tag="w16")
                        nc.scalar.activation(
                            cand[:, :wd], s[:, :wd], AF.Tanh,
                            bias=bc_sb[:, j : j + 1],
                            scale=INV if j <= 1 else 1.0,
                        )
                        m = wpool.tile([P, 512], F16, tag="w16")
                        nc.vector.tensor_mul(
                            m[:, :wd], z_sb[hb][:, lo : lo + wd], cand[:, :wd]
                        )
                        o_sb = wpool.tile([P, 512], F16, tag="w16")
                        nc.vector.tensor_sub(o_sb[:, :wd], m[:, :wd], zh[hb][:, lo : lo + wd])
                        nc.scalar.dma_start(
                            outT[:, ooff + lo : ooff + lo + wd], o_sb[:, :wd]
                        )

                    if last:
                        blend(0, 256, pap=pxs[0], pofs=0)
                        blend(256, 256, pap=pxs[1], pofs=0)
                    else:
                        blend(0, 512)

    nc.compile()
    return nc


def _pack_acts(a, dtype, scale=1.0):
    # [BL, H] -> [p, hb*4096 + kc*512 + bb] with scale*a[hb*512+bb, kc*128+p]
    t = (np.asarray(a, np.float32) * scale).reshape(2, 512, KC, P)
    t = np.ascontiguousarray(t.transpose(3, 0, 2, 1).reshape(P, 2 * HB))
    if dtype is E4M3:
        t = np.clip(t, -240.0, 240.0)
    return t.astype(dtype)


def _pack_weights(W_ih, b_ih, W_hh, b_hh, W_c, b_c, W_hc, b_hc):
    Wg_full = np.concatenate([W_ih, W_hh], axis=0) * SW  # [2H, 2H] = [k, o]
    Wg8H = np.ascontiguousarray(
        Wg_full.reshape(16, P, 16, P).transpose(1, 2, 0, 3).reshape(P, 16 * 2048)
    ).astype(E4M3)
    Wc16H = np.ascontiguousarray(
        W_c.reshape(KC, P, NJ, P).transpose(1, 2, 0, 3).reshape(P, NJ * H)
    ).astype(np.float16)
    Whc8H = np.ascontiguousarray(
        (W_hc * SW).reshape(KC, P, NJ, P).transpose(1, 2, 0, 3).reshape(P, NJ * H)
    ).astype(E4M3)
    Wc8loH = np.ascontiguousarray(
        (W_c[:, : 2 * P] * SW).reshape(KC, P, 2, P).transpose(1, 2, 0, 3).reshape(P, 2 * H)
    ).astype(E4M3)
    bgH = np.ascontiguousarray((b_ih + b_hh).reshape(16, P).T).astype(np.float32)
    bcH = np.ascontiguousarray(b_c.reshape(NJ, P).T).astype(np.float32)
    bhcsH = np.ascontiguousarray((b_hc / INV).reshape(NJ, P).T).astype(np.float32)
    return Wg8H, Wc16H, Whc8H, Wc8loH, bgH, bcH, bhcsH


def _make_in_maps(input, hx, W_ih, b_ih, W_hh, b_hh, W_c, b_c, W_hc, b_hc):
    Wg8H, Wc16H, Whc8H, Wc8loH, bgH, bcH, bhcsH = _pack_weights(
        np.asarray(W_ih, np.float32), np.asarray(b_ih, np.float32),
        np.asarray(W_hh, np.float32), np.asarray(b_hh, np.float32),
        np.asarray(W_c, np.float32), np.asarray(b_c, np.float32),
        np.asarray(W_hc, np.float32), np.asarray(b_hc, np.float32),
    )
    in_maps = []
    for i in range(N_CORES):
        xs = np.asarray(input, np.float32)[i * BL : (i + 1) * BL]
        hs = np.asarray(hx, np.float32)[i * BL : (i + 1) * BL]
        in_maps.append(
            {
                "x8": _pack_acts(xs, E4M3, SA),
                "h8": _pack_acts(hs, E4M3, SA),
                "x16": _pack_acts(xs, np.float16),
                "h16": _pack_acts(hs, np.float16),
                "Wg8": Wg8H,
                "Whc8": Whc8H,
                "Wc16": Wc16H,
                "Wc8lo": Wc8loH,
                "bg": bgH,
                "bc": bcH,
                "bhcs": bhcsH,
            }
        )
    return in_maps


def kernel(input, hx, W_ih, b_ih, W_hh, b_hh, W_c, b_c, W_hc, b_hc):
    if "nc" not in _CACHE:
        _CACHE["nc"] = _build_program()
    nc = _CACHE["nc"]

    in_maps = _make_in_maps(
        input, hx, W_ih, b_ih, W_hh, b_hh, W_c, b_c, W_hc, b_hc
    )
    res = run_bass_kernel_spmd(nc, in_maps, core_ids=list(range(N_CORES)))
    out = np.empty((B, H), np.float32)
    for i, r in enumerate(res.results):
        o = (
            r["outT"].astype(np.float32)
            .reshape(P, NJ, 2, 512).transpose(2, 3, 1, 0).reshape(BL, H)
        )
        out[i * BL : (i + 1) * BL] = o
    return out
